# revision 7
# baseline (speedup 1.0000x reference)
"""Multi-head attention forward (B=2, N=2048, C=768, H=12) on 8 TRN2 cores.

Sharding: core = b*4 + g handles batch b, heads 3g..3g+2 (tensor parallel on
heads). Each core computes qkv for its heads, flash-style attention with the
full N x N logits kept on-chip (transposed [m, nq] layout so softmax's key
mask folds into the exp bias and the denominator comes from a ones-column in
V), and a partial output projection over its 192 channels. Host sums the 4
partials per batch and adds the bias.

All matmuls run in float32r (TF32-like, 1 cycle/row) for ~4x over fp32.
"""

import numpy as np

from concourse import bacc
import concourse.mybir as mybir
import concourse.tile as tile
from concourse.bass_utils import run_bass_kernel_spmd

B, N, C = 2, 2048, 768
H, DH = 12, 64
G = 4          # head groups (cores per batch)
HPC = 3        # heads per core
P = 128
KT = C // P    # 6 contraction tiles over channels
NMT = N // P   # 16 key (m) tiles
NQT = N // P   # 16 query tiles
W = 1024       # nq chunk width for logits/exp
NH = N // W    # 2 nq chunks
VBLK = 2 * DH  # 128: per-(m,head) block in v_sb = [v (64) | ones (64)]

TRACE = False
LAST_EXEC_NS = None
LAST_RESULTS = None

_nc_cache = {}

f32 = mybir.dt.float32
f32r = mybir.dt.float32r

_VONES = np.ones((P, NMT * HPC * DH), np.float32)


def _build(reps=1):
    nc = bacc.Bacc("TRN2", debug=False)

    xT = nc.dram_tensor("xT", [C, N], f32r, kind="ExternalInput")
    wqkT = nc.dram_tensor("wqkT", [C, 6 * DH], f32r, kind="ExternalInput")
    wvT = nc.dram_tensor("wvT", [C, HPC * VBLK], f32r, kind="ExternalInput")
    wpT = nc.dram_tensor("wpT", [HPC * DH, C], f32r, kind="ExternalInput")
    mbias = nc.dram_tensor("mbias", [P, NMT], f32, kind="ExternalInput")
    vones = nc.dram_tensor("vones", [P, NMT * HPC * DH], f32r, kind="ExternalInput")
    y = nc.dram_tensor("y", [N, C], f32, kind="ExternalOutput")

    with tile.TileContext(nc) as tc:
        with (
            tc.tile_pool(name="big", bufs=1) as big,
            tc.tile_pool(name="exps", bufs=3) as exps,
            tc.tile_pool(name="recips", bufs=2) as recips,
            tc.tile_pool(name="ys", bufs=3) as ys,
            tc.tile_pool(name="pa", bufs=2, space="PSUM") as pa,
            tc.tile_pool(name="pb", bufs=2, space="PSUM") as pb,
        ):
            body(nc, tc, big, exps, recips, ys, pa, pb,
                 xT, wqkT, wvT, wpT, mbias, vones, y, reps)

    nc.compile()
    return nc


def body(nc, tc, big, exps, recips, ys, pa, pb, xT, wqkT, wvT, wpT, mbias, vones, y, reps):
    QCOLS = HPC * DH
    for _rep in range(reps):
            xT_sb = big.tile([P, KT * N], f32r, tag="xT")
            wqk_sb = big.tile([P, KT * 6 * DH], f32r, tag="wqk")
            wv_sb = big.tile([P, KT * HPC * VBLK], f32r, tag="wv")
            wp_sb = [big.tile([DH, C], f32r, tag=f"wp{h}", name=f"wp{h}") for h in range(HPC)]
            mb_sb = big.tile([P, NMT], f32, tag="mb")
            ones_sb = big.tile([P, DH], f32r, tag="ones")
            # T1: q/k for heads 0 (parts 0:64) and 1 (parts 64:128)
            # T2: q/k for head 2 (parts 0:64)
            t1 = big.tile([P, 2 * N], f32r, tag="t1")
            t2 = big.tile([P, 2 * N], f32r, tag="t2")
            v_sb = big.tile([P, NMT * HPC * VBLK], f32r, tag="v")
            at_sb = [big.tile([DH, N], f32r, tag=f"at{h}", name=f"at{h}") for h in range(HPC)]

            # --- input DMAs ---
            nc.sync.dma_start(mb_sb[:], mbias[:, :])
            for k in range(KT):
                nc.sync.dma_start(
                    xT_sb[:, k * N : (k + 1) * N], xT[k * P : (k + 1) * P, :]
                )
            for k in range(KT):
                nc.sync.dma_start(
                    wqk_sb[:, k * 6 * DH : (k + 1) * 6 * DH],
                    wqkT[k * P : (k + 1) * P, :],
                )
            for k in range(KT):
                nc.sync.dma_start(
                    wv_sb[:, k * HPC * VBLK : (k + 1) * HPC * VBLK],
                    wvT[k * P : (k + 1) * P, :],
                )
            for h in range(HPC):
                nc.sync.dma_start(wp_sb[h][:], wpT[h * DH : (h + 1) * DH, :])
            nc.sync.dma_start(ones_sb[:], vones[:, 0:DH])
            # ones columns of v_sb (cols 64:128 of every 128-block)
            v_ones = v_sb[:].rearrange("p (b x) -> p b x", x=VBLK)[:, :, DH:VBLK]
            nc.sync.dma_start(v_ones, vones[:, :])

            # --- qT/kT: out rows = head dims, cols = n (d-major) ---
            # passes: (lhsT col range in wqkT, M, dest tile, dest col offset)
            qk_passes = [
                (0, P, t1, 0),            # q heads 0,1
                (QCOLS, P, t1, N),        # k heads 0,1
                (2 * DH, DH, t2, 0),      # q head 2
                (QCOLS + 2 * DH, DH, t2, N),  # k head 2
            ]
            for c0, m, dest, dcol in qk_passes:
                for ch in range(N // W):
                    ps = pa.tile([P, W], f32, tag="pa")
                    for s in range(W // 512):
                        for k in range(KT):
                            nc.tensor.matmul(
                                ps[:m, s * 512 : (s + 1) * 512],
                                wqk_sb[:, k * 6 * DH + c0 : k * 6 * DH + c0 + m],
                                xT_sb[
                                    :,
                                    k * N + ch * W + s * 512 : k * N
                                    + ch * W
                                    + (s + 1) * 512,
                                ],
                                start=(k == 0),
                                stop=(k == KT - 1),
                            )
                    nc.vector.tensor_copy(
                        dest[:m, dcol + ch * W : dcol + (ch + 1) * W], ps[:m, :]
                    )

            # --- v in natural [m, d] layout, blocks [v_h | ones] per head ---
            VB = HPC * VBLK  # 384
            for mt in range(NMT):
                ps = pb.tile([P, W], f32, tag="pb")
                for k in range(KT):
                    nc.tensor.matmul(
                        ps[:, :VB],
                        xT_sb[:, k * N + mt * P : k * N + (mt + 1) * P],
                        wv_sb[:, k * VB : (k + 1) * VB],
                        start=(k == 0),
                        stop=(k == KT - 1),
                    )
                nc.vector.tensor_copy(
                    v_sb[:].rearrange("p (b x) -> p b x", x=VBLK)[
                        :, mt * HPC : (mt + 1) * HPC, 0:DH
                    ],
                    ps[:, :VB].rearrange("p (b x) -> p b x", x=VBLK)[:, :, 0:DH],
                )

            # --- attention per (nq half, head) ---
            for half in range(NH):
                for h in range(HPC):
                    if h == 0:
                        qk, prow = t1, 0
                    elif h == 1:
                        qk, prow = t1, DH
                    else:
                        qk, prow = t2, 0
                    ps_pv = pb.tile([P, W], f32, tag="pb")
                    for mt in range(NMT):
                        ps_l = pa.tile([P, W], f32, tag="pa")
                        for s in range(W // 512):
                            nc.tensor.matmul(
                                ps_l[:, s * 512 : (s + 1) * 512],
                                qk[prow : prow + DH, N + mt * P : N + (mt + 1) * P],
                                qk[
                                    prow : prow + DH,
                                    half * W + s * 512 : half * W + (s + 1) * 512,
                                ],
                                start=True,
                                stop=True,
                            )
                        et = exps.tile([P, W], f32r, tag="exp")
                        nc.scalar.activation(
                            et[:],
                            ps_l[:],
                            mybir.ActivationFunctionType.Exp,
                            bias=mb_sb[:, mt : mt + 1],
                            scale=float(DH) ** -0.5,
                        )
                        for s in range(W // 512):
                            nc.tensor.matmul(
                                ps_pv[: DH + 1, s * 512 : (s + 1) * 512],
                                v_sb[
                                    :,
                                    (mt * HPC + h) * VBLK : (mt * HPC + h) * VBLK
                                    + DH
                                    + 1,
                                ],
                                et[:, s * 512 : (s + 1) * 512],
                                start=(mt == 0),
                                stop=(mt == NMT - 1),
                            )
                    # rows 0:64 = unnormalized out^T, row 64 = softmax denom
                    rc = recips.tile([P, W], f32r, tag="rc")
                    with nc.allow_low_precision(reason="f32r softmax denom"):
                        nc.vector.reciprocal(
                            rc[DH : DH + 1, :], ps_pv[DH : DH + 1, :]
                        )
                    ps_rb = pb.tile([P, W], f32, tag="pb")
                    for s in range(W // 512):
                        nc.tensor.matmul(
                            ps_rb[:DH, s * 512 : (s + 1) * 512],
                            ones_sb[DH : DH + 1, :],
                            rc[DH : DH + 1, s * 512 : (s + 1) * 512],
                            start=True,
                            stop=True,
                        )
                    rb_sb = recips.tile([DH, W], f32, tag="rb", name="rb")
                    nc.scalar.copy(rb_sb[:], ps_rb[:DH, :])
                    nc.vector.tensor_mul(
                        at_sb[h][:, half * W : (half + 1) * W],
                        ps_pv[:DH, :],
                        rb_sb[:],
                    )

                # --- projection for this half's nq tiles ---
                for nt in range(half * NQT // NH, (half + 1) * NQT // NH):
                    ps_y = pa.tile([P, W], f32, tag="pa")
                    for o0, ow in ((0, 512), (512, 256)):
                        for h in range(HPC):
                            nc.tensor.matmul(
                                ps_y[:, o0 : o0 + ow],
                                at_sb[h][:, nt * P : (nt + 1) * P],
                                wp_sb[h][:, o0 : o0 + ow],
                                start=(h == 0),
                                stop=(h == HPC - 1),
                            )
                    yt = ys.tile([P, C], f32, tag="y")
                    nc.vector.tensor_copy(yt[:], ps_y[:, :C])
                    nc.sync.dma_start(y[nt * P : (nt + 1) * P, :], yt[:])


def _get_nc(reps=1):
    if reps not in _nc_cache:
        _nc_cache[reps] = _build(reps)
    return _nc_cache[reps]


def kernel(x, att_mask, qkv_w, proj_w, proj_b):
    global LAST_EXEC_NS, LAST_RESULTS
    x = np.asarray(x, dtype=np.float32)
    att_mask = np.asarray(att_mask)
    qkv_w = np.asarray(qkv_w, dtype=np.float32)
    proj_w = np.asarray(proj_w, dtype=np.float32)
    proj_b = np.asarray(proj_b, dtype=np.float32)

    nc = _get_nc()

    in_maps = []
    for b in range(B):
        xT = np.ascontiguousarray(x[b].T)
        mb = np.where(att_mask[b] == 0, -1e30, 0.0).astype(np.float32)
        mbias = np.ascontiguousarray(mb.reshape(NMT, P).T)
        for g in range(G):
            r0 = g * HPC * DH
            r1 = (g + 1) * HPC * DH
            wq = qkv_w[r0:r1]                # [192, 768]
            wk = qkv_w[C + r0 : C + r1]
            wv = qkv_w[2 * C + r0 : 2 * C + r1]
            wqkT = np.ascontiguousarray(np.concatenate([wq, wk], 0).T)
            wvT = np.zeros((C, HPC * VBLK), np.float32)
            for h in range(HPC):
                wvT[:, h * VBLK : h * VBLK + DH] = wv[h * DH : (h + 1) * DH].T
            wpT = np.ascontiguousarray(proj_w[:, r0:r1].T)
            in_maps.append(
                {
                    "xT": xT,
                    "wqkT": wqkT,
                    "wvT": wvT,
                    "wpT": wpT,
                    "mbias": mbias,
                    "vones": _VONES,
                }
            )

    res = run_bass_kernel_spmd(
        nc, in_maps, core_ids=list(range(B * G)), trace=TRACE
    )
    LAST_EXEC_NS = res.exec_time_ns
    LAST_RESULTS = res

    out = np.zeros((B, N, C), np.float32)
    for b in range(B):
        acc = res.results[b * G]["y"].copy()
        for g in range(1, G):
            acc += res.results[b * G + g]["y"]
        out[b] = acc + proj_b[None, :]
    return out
